# revision 20
# baseline (speedup 1.0000x reference)
"""Multi-head attention block (QKV proj + SDPA + merge-scramble + fc +
residual + LayerNorm) on 8 Trainium2 NeuronCores.

Sharding: data-parallel over the flattened batch dim (b*n = 32 sequences),
4 sequences per core. Each core runs an identical Bass program on its shard.

v5 = HW-measured optimum of the fp8 design space:
 - Q/K/V projections and the fc matmul run as fp8(e4m3) DoubleRow matmuls
   (2 contraction subtiles per instruction; measured ~1.6x faster than
   bf16 on HW). Inputs q/k/v and weights are quantized to fp8 on the host.
 - S^T = K Q^T stays bf16 (precision: fp8 Q/K noise dominated the error
   budget) and exp keeps its bf16 output (measured: fp8 ACT output costs
   +1.4us per [128,1024] exp -- ~90us/core, a disaster).
 - AV stays bf16 non-DoubleRow (DR would require fp8 expS).
 - x (the attention output) is written as fp8 by the AV-normalize (free on
   DVE) to feed the fc DoubleRow. The V_aug ones-column is 0.125 so the
   normalize yields x = 8*O, keeping x in e4m3's normal range; the
   residual qn is pre-scaled by 8 on the host (LayerNorm is
   scale-invariant).
 - Output is fp16 (halves the output DMA); host converts back to fp32.
 - Weight DMAs are hoisted out of the steady-state loop.
 - Emission is ACT-first software pipelining: the S^T head-pair groups of
   seq s (feeding ScalarE's exps) interleave with AV/fc/LN of s-1 and the
   projections of s+1, and carry top scheduler priority so fc/proj
   matmuls never head-of-line-block a ready S^T in the PE queue.
"""

import numpy as np

import concourse.bacc as bacc
import concourse.mybir as mybir
import concourse.tile as tile
from concourse.bass_utils import run_bass_kernel_spmd

F32 = mybir.dt.float32
F16 = mybir.dt.float16
I32 = mybir.dt.int32
BF16 = mybir.dt.bfloat16
F8 = mybir.dt.float8e4
AF = mybir.ActivationFunctionType
OP = mybir.AluOpType
DR = mybir.MatmulPerfMode.DoubleRow

N_CORES = 8
S = 4          # sequences per core
T = 512        # sequence length
D = 512        # model dim (= e = n_head * d_k)
NH = 8         # heads
DV = 64        # head dim
C = 4          # 128-row chunks per 512 dim
P = 128
EPS = 1e-6

_PROGRAM_CACHE = {}


def _build_program(apply_affine: bool, loop_iters: int = 1):
    nc = bacc.Bacc()

    # inputs are host-repacked to [.., P, C*T]: each partition's whole
    # working set is contiguous in DRAM.
    qT = nc.declare_dram_parameter("qT", [S, P, C * T], F8, isOutput=False)
    kT = nc.declare_dram_parameter("kT", [S, P, C * T], F8, isOutput=False)
    vT = nc.declare_dram_parameter("vT", [S, P, C * T], F8, isOutput=False)
    qn = nc.declare_dram_parameter("qn", [S, P, C * D], BF16, isOutput=False)
    wq = nc.declare_dram_parameter("wq", [P, C * D], F8, isOutput=False)
    wk = nc.declare_dram_parameter("wk", [P, C * D], F8, isOutput=False)
    wv = nc.declare_dram_parameter("wv", [P, C * D], F8, isOutput=False)
    wfc = nc.declare_dram_parameter("wfc", [P, C * D], F8, isOutput=False)
    if apply_affine:
        gmb = nc.declare_dram_parameter("gmb", [P, D], F32, isOutput=False)
        btb = nc.declare_dram_parameter("btb", [P, D], F32, isOutput=False)
    out = nc.declare_dram_parameter("out", [S, T, D], BF16, isOutput=True)

    with tile.TileContext(nc) as tc:
        with (
            tc.tile_pool(name="const", bufs=1) as cst,
            tc.tile_pool(name="inp", bufs=3) as inp,
            tc.tile_pool(name="proj", bufs=2) as proj,
            tc.tile_pool(name="expp", bufs=10) as expp,
            tc.tile_pool(name="xp", bufs=3) as xp,
            tc.tile_pool(name="small", bufs=2) as small,
            tc.tile_pool(name="psc", bufs=2, space="PSUM") as psc,
            tc.tile_pool(name="pfc", bufs=2, space="PSUM") as pfc,
            tc.tile_pool(name="pav", bufs=2, space="PSUM") as pavp,
        ):
            wq_sb = cst.tile([P, C, D], F8, tag="wq")
            wk_sb = cst.tile([P, C, D], F8, tag="wk")
            wv_sb = cst.tile([P, C, D], F8, tag="wv")
            wfc_sb = cst.tile([P, C, D], F8, tag="wfc")
            magic_sb = cst.tile([P, 1], I32, tag="magic")
            nc.vector.memset(magic_sb[:], 0x5F3759DF)
            if apply_affine:
                gm_sb = cst.tile([P, D], F32, tag="gmb")
                bt_sb = cst.tile([P, D], F32, tag="btb")
                nc.sync.dma_start(gm_sb[:], gmb[:])
                nc.sync.dma_start(bt_sb[:], btb[:])

            # weights loaded ONCE (outside the steady-state loop)
            for w_sb, w in ((wq_sb, wq), (wk_sb, wk), (wv_sb, wv),
                            (wfc_sb, wfc)):
                nc.sync.dma_start(w_sb.rearrange("p c e -> p (c e)"), w[:])

            def load(s):
                st = {}
                st["qT"] = inp.tile([P, C, T], F8, tag="qT", name="qT_sb")
                st["kT"] = inp.tile([P, C, T], F8, tag="kT", name="kT_sb")
                st["vT"] = inp.tile([P, C, T], F8, tag="vT", name="vT_sb")
                for sb, dr in ((st["qT"], qT), (st["kT"], kT),
                               (st["vT"], vT)):
                    nc.sync.dma_start(
                        sb.rearrange("p c t -> p (c t)"), dr[s]
                    )
                st["qn"] = inp.tile([P, C, D], BF16, tag="qnf", name="qn_sb")
                nc.sync.dma_start(
                    st["qn"].rearrange("p c d -> p (c d)"), qn[s]
                )
                return st

            def projQK(s, st, ecs):
                # Q^T/K^T [e, t] head-major bf16; fp8 DoubleRow matmuls,
                # DVE drains. Chunk ec feeds attnB_group hp=ec.
                if ecs[0] == 0:
                    st["QT"] = proj.tile([P, C, T], BF16, tag="QT",
                                         name="QT_sb")
                    st["KT"] = proj.tile([P, C, T], BF16, tag="KT",
                                         name="KT_sb")
                for ec in ecs:
                    for dst, w_sb, x_sb in (
                        (st["QT"], wq_sb, st["qT"]),
                        (st["KT"], wk_sb, st["kT"]),
                    ):
                        ps = pfc.tile([P, T], F32, tag="fc", name="ps")
                        for dc in range(0, C, 2):
                            nc.tensor.matmul(
                                ps[:],
                                lhsT=w_sb[:, dc:dc + 2, ec * P:(ec + 1) * P],
                                rhs=x_sb[:, dc:dc + 2, :],
                                start=(dc == 0),
                                stop=(dc == C - 2),
                                perf_mode=DR,
                            )
                        nc.vector.tensor_copy(dst[:, ec, :], ps[:])

            def projV(s, st):
                # V [t, e] bf16 with per-head 0.125 column (the AV ones
                # column is 0.125 so x comes out as 8*O for fp8 range)
                V_sb = proj.tile([P, C, NH, DV + 1], BF16, tag="V",
                                 name="V_sb")
                st["V"] = V_sb
                nc.gpsimd.memset(V_sb[:, :, :, DV:DV + 1], 0.125)
                for tc_ in range(C):
                    ps = pfc.tile([P, T], F32, tag="fc", name="ps")
                    for dc in range(0, C, 2):
                        nc.tensor.matmul(
                            ps[:],
                            lhsT=st["vT"][:, dc:dc + 2, tc_ * P:(tc_ + 1) * P],
                            rhs=wv_sb[:, dc:dc + 2, :],
                            start=(dc == 0),
                            stop=(dc == C - 2),
                            perf_mode=DR,
                        )
                    eng = nc.scalar.copy if tc_ < 2 else nc.vector.tensor_copy
                    eng(
                        V_sb[:, tc_, :, 0:DV],
                        ps.rearrange("p (h v) -> p h v", h=NH),
                    )

            def attnB_group(s, st, hp):
                # S^T = K_h Q_h^T for one head pair, bf16; exp(S/8) on
                # ScalarE into bf16 expS tiles. Head pairs share a 2-bank
                # psum tile so one [128,1024] exp drains them. The chain
                # gets top scheduler priority: ScalarE is the binding
                # engine and must never wait behind fc/proj PE work.
                if hp == 0:
                    st["expS"] = []
                eP = expp.tile([P, C, 2, T], BF16, tag="expS", name="expSp")
                st["expS"] += [eP[:, :, 0, :], eP[:, :, 1, :]]
                with tc.high_priority():
                    for tkc in range(C):
                        ps2 = psc.tile([P, 2, T], F32, tag="sc", name="ps2")
                        for sub in range(2):
                            nc.tensor.matmul(
                                ps2[:, sub, :],
                                lhsT=st["KT"][sub * DV:(sub + 1) * DV, hp,
                                              tkc * P:(tkc + 1) * P],
                                rhs=st["QT"][sub * DV:(sub + 1) * DV, hp, :],
                                start=True,
                                stop=True,
                            )
                        nc.scalar.activation(
                            eP[:, tkc, :, :], ps2[:], AF.Exp, scale=0.125,
                        )

            def avH(s, st, half):
                # O-form AV for heads [4*half, 4*half+4): bf16 matmuls; col
                # 64 of each head is sum(w)/8, so the reciprocal-normalize
                # yields x = 8*O, written as fp8 (feeds the fc DoubleRow).
                if half == 0:
                    st["x"] = xp.tile([P, C, T], F8, tag="x", name="x_sb")
                x_sb = st["x"]
                W = DV + 1
                for tqc in range(C):
                    pv = pavp.tile([P, 4 * W], F32, tag="av", name="pav")
                    for hh in range(4):
                        h = 4 * half + hh
                        col = hh * W
                        for tkc in range(C):
                            nc.tensor.matmul(
                                pv[:, col:col + W],
                                lhsT=st["expS"][h][:, tkc,
                                                   tqc * P:(tqc + 1) * P],
                                rhs=st["V"][:, tkc, h, :],
                                start=(tkc == 0),
                                stop=(tkc == C - 1),
                            )
                    rc = small.tile([P, 4], F32, tag="rc", bufs=4, name="rc")
                    nc.vector.reciprocal(rc[:], pv[:, DV:4 * W:W])
                    nc.vector.tensor_tensor(
                        x_sb[:, tqc, half * 256:(half + 1) * 256]
                            .rearrange("p (h v) -> p h v", h=4),
                        pv.rearrange("p (h x) -> p h x", h=4)[:, :, 0:DV],
                        rc[:, :, None].to_broadcast((P, 4, DV)),
                        OP.mult,
                    )

            def tailC_fc(s, st):
                # fc (contracting over the *time* index, thanks to the
                # reference's transpose-view scramble) as fp8 DoubleRow +
                # residual add + LN stats
                st2_seq = small.tile([P, C, 2], F32, tag="st2",
                                     name="st2_seq")
                y_sb = small.tile([P, C, D], F32, tag="y", bufs=2,
                                  name="y_sb")
                y16 = small.tile([P, C, D], BF16, tag="y16", bufs=2,
                                 name="y16")
                st["st2"], st["y"], st["y16"] = st2_seq, y_sb, y16
                for ac in range(C):
                    psy = pfc.tile([P, T], F32, tag="fc", name="psy")
                    for cc in range(0, C, 2):
                        nc.tensor.matmul(
                            psy[:],
                            lhsT=st["x"][:, cc:cc + 2, ac * P:(ac + 1) * P],
                            rhs=wfc_sb[:, cc:cc + 2, :],
                            start=(cc == 0),
                            stop=(cc == C - 2),
                            perf_mode=DR,
                        )
                    nc.vector.tensor_tensor(
                        y_sb[:, ac, :], psy[:], st["qn"][:, ac, :], OP.add
                    )
                    st6 = small.tile([P, 6], F32, tag="st6", name="st6")
                    nc.vector.bn_stats(st6[:], y_sb[:, ac, :])
                    nc.vector.bn_aggr(st2_seq[:, ac, :], st6[:])

            def tailC_ln(s, st):
                st2_seq, y_sb, y16 = st["st2"], st["y"], st["y16"]
                # rinv = rsqrt(var) via the bit-hack seed + 2 Newton
                # iterations, entirely on DVE (max rel err ~5e-6). Keeps
                # Exp as the ONLY ScalarE table.
                rinv = small.tile([P, C], F32, tag="rinv", name="rinv")
                t1 = small.tile([P, C], F32, tag="nt1", name="t1")
                t2 = small.tile([P, C], F32, tag="nt2", name="t2")
                var_i = st2_seq.bitcast(I32)[:, :, 1]
                nc.vector.tensor_scalar(
                    t1.bitcast(I32)[:], var_i, 1, None, OP.arith_shift_right
                )
                nc.vector.tensor_tensor(
                    rinv.bitcast(I32)[:],
                    magic_sb[:].to_broadcast((P, C)),
                    t1.bitcast(I32)[:],
                    OP.subtract,
                )
                for _ in range(2):
                    nc.vector.tensor_tensor(t1[:], rinv[:], rinv[:], OP.mult)
                    nc.vector.tensor_tensor(
                        t2[:], t1[:], st2_seq[:, :, 1], OP.mult
                    )
                    nc.vector.tensor_scalar(
                        t2[:], t2[:], -0.5, 1.5, OP.mult, OP.add
                    )
                    nc.vector.tensor_tensor(rinv[:], rinv[:], t2[:], OP.mult)
                for ac in range(C):
                    if apply_affine:
                        nc.vector.tensor_scalar(
                            y_sb[:, ac, :], y_sb[:, ac, :],
                            st2_seq[:, ac, 0:1], rinv[:, ac:ac + 1],
                            OP.subtract, OP.mult,
                        )
                        nc.vector.tensor_tensor(
                            y_sb[:, ac, :], y_sb[:, ac, :], gm_sb[:], OP.mult
                        )
                        nc.vector.tensor_tensor(
                            y16[:, ac, :], y_sb[:, ac, :], bt_sb[:], OP.add
                        )
                    else:
                        nc.vector.tensor_scalar(
                            y16[:, ac, :], y_sb[:, ac, :],
                            st2_seq[:, ac, 0:1], rinv[:, ac:ac + 1],
                            OP.subtract, OP.mult,
                        )
                # out on the ACT HWDGE ring keeps the SP queue free for
                # the next body's input loads.
                for ac in range(C):
                    nc.scalar.dma_start(
                        out[s, ac * P:(ac + 1) * P, :], y16[:, ac, :]
                    )

            # ACT-first software pipelining (see module docstring)
            def emit_all():
                sts = {}
                sts[0] = load(0)
                sts[1] = load(1)
                projQK(0, sts[0], (0, 1))
                projQK(0, sts[0], (2, 3))
                for s in range(S):
                    if s + 2 < S:
                        sts[s + 2] = load(s + 2)
                    attnB_group(s, sts[s], 0)
                    projV(s, sts[s])
                    if s > 0:
                        avH(s - 1, sts[s - 1], 0)
                    attnB_group(s, sts[s], 1)
                    if s > 0:
                        avH(s - 1, sts[s - 1], 1)
                    attnB_group(s, sts[s], 2)
                    if s > 0:
                        tailC_fc(s - 1, sts[s - 1])
                    if s + 1 < S:
                        projQK(s + 1, sts[s + 1], (0, 1))
                    attnB_group(s, sts[s], 3)
                    if s + 1 < S:
                        projQK(s + 1, sts[s + 1], (2, 3))
                    if s > 0:
                        tailC_ln(s - 1, sts[s - 1])
                avH(S - 1, sts[S - 1], 0)
                avH(S - 1, sts[S - 1], 1)
                tailC_fc(S - 1, sts[S - 1])
                tailC_ln(S - 1, sts[S - 1])

            if loop_iters == 1:
                emit_all()
            else:
                with tc.For_i(0, loop_iters, 1):
                    emit_all()

    nc.finalize()
    return nc


def _get_program(apply_affine: bool, loop_iters: int = 1):
    key = (apply_affine, loop_iters)
    if key not in _PROGRAM_CACHE:
        _PROGRAM_CACHE[key] = _build_program(apply_affine, loop_iters)
    return _PROGRAM_CACHE[key]


def _pack(a, dtype):
    # [.., C*P, F] -> [.., P, C*F] so each partition row is contiguous
    sh = a.shape[:-2]
    cp, f = a.shape[-2], a.shape[-1]
    return np.ascontiguousarray(
        a.reshape(*sh, C, P, f).swapaxes(-3, -2).reshape(*sh, P, C * f)
    ).astype(dtype)


def kernel(q, k, v, w_q, w_k, w_v, w_fc, ln_gamma, ln_beta, _res_holder=None):
    q = np.asarray(q, dtype=np.float32)
    k = np.asarray(k, dtype=np.float32)
    v = np.asarray(v, dtype=np.float32)
    w_q = np.asarray(w_q, dtype=np.float32)
    w_k = np.asarray(w_k, dtype=np.float32)
    w_v = np.asarray(w_v, dtype=np.float32)
    w_fc = np.asarray(w_fc, dtype=np.float32)
    ln_gamma = np.asarray(ln_gamma, dtype=np.float32)
    ln_beta = np.asarray(ln_beta, dtype=np.float32)

    b, n, t, d = q.shape
    B = b * n
    assert (b, n, t, d) == (8, 4, T, D), q.shape
    qf = q.reshape(B, t, d)
    kf = k.reshape(B, t, d)
    vf = v.reshape(B, t, d)

    apply_affine = not (
        np.all(ln_gamma == 1.0) and np.all(ln_beta == 0.0)
    )
    nc = _get_program(apply_affine)

    bf16 = mybir.dt.np(BF16)
    f8 = mybir.dt.np(F8)

    wq_p = _pack(w_q.T, f8)
    wk_p = _pack(w_k.T, f8)
    wv_p = _pack(w_v.T, f8)
    wfc_p = _pack(w_fc.T, f8)

    in_maps = []
    for c in range(N_CORES):
        sl = slice(S * c, S * (c + 1))
        m = {
            "qT": _pack(qf[sl].transpose(0, 2, 1), f8),
            "kT": _pack(kf[sl].transpose(0, 2, 1), f8),
            "vT": _pack(vf[sl].transpose(0, 2, 1), f8),
            "qn": _pack(qf[sl] * 8.0, bf16),
            "wq": wq_p, "wk": wk_p, "wv": wv_p, "wfc": wfc_p,
        }
        if apply_affine:
            m["gmb"] = np.ascontiguousarray(
                np.broadcast_to(ln_gamma, (P, D)).astype(np.float32)
            )
            m["btb"] = np.ascontiguousarray(
                np.broadcast_to(ln_beta, (P, D)).astype(np.float32)
            )
        in_maps.append(m)

    res = run_bass_kernel_spmd(nc, in_maps, list(range(N_CORES)))
    if _res_holder is not None:
        _res_holder.append(res)
    full = np.concatenate(
        [res.results[c]["out"].astype(np.float32) for c in range(N_CORES)],
        axis=0,
    )
    return full.reshape(b, n, t, d)


def prep_per_core(inputs):
    qf = inputs["q"].reshape(32, 512, 512).astype(np.float32)
    kf = inputs["k"].reshape(32, 512, 512).astype(np.float32)
    vf = inputs["v"].reshape(32, 512, 512).astype(np.float32)
    bf16 = mybir.dt.np(BF16)
    f8 = mybir.dt.np(F8)

    wq_p = _pack(inputs["w_q"].T.astype(np.float32), f8)
    wk_p = _pack(inputs["w_k"].T.astype(np.float32), f8)
    wv_p = _pack(inputs["w_v"].T.astype(np.float32), f8)
    wfc_p = _pack(inputs["w_fc"].T.astype(np.float32), f8)

    per_core = []
    for c in range(N_CORES):
        sl = slice(S * c, S * (c + 1))
        per_core.append({
            "qT": _pack(qf[sl].transpose(0, 2, 1), f8),
            "kT": _pack(kf[sl].transpose(0, 2, 1), f8),
            "vT": _pack(vf[sl].transpose(0, 2, 1), f8),
            "qn": _pack(qf[sl] * 8.0, bf16),
            "wq": wq_p, "wk": wk_p, "wv": wv_p, "wfc": wfc_p,
        })
    return per_core


# revision 21
# speedup vs baseline: 1.4501x; 1.4501x over previous
"""Multi-head attention block (QKV proj + SDPA + merge-scramble + fc +
residual + LayerNorm) on 8 Trainium2 NeuronCores.

Sharding: data-parallel over the flattened batch dim (b*n = 32 sequences),
4 sequences per core. Each core runs an identical Bass program on its shard.

v5 = HW-measured optimum of the fp8 design space:
 - Q/K/V projections and the fc matmul run as fp8(e4m3) DoubleRow matmuls
   (2 contraction subtiles per instruction; measured ~1.6x faster than
   bf16 on HW). Inputs q/k/v and weights are quantized to fp8 on the host.
 - S^T = K Q^T stays bf16 (precision: fp8 Q/K noise dominated the error
   budget) and exp keeps its bf16 output (measured: fp8 ACT output costs
   +1.4us per [128,1024] exp -- ~90us/core, a disaster).
 - AV stays bf16 non-DoubleRow (DR would require fp8 expS).
 - x (the attention output) is written as fp8 by the AV-normalize (free on
   DVE) to feed the fc DoubleRow. The V_aug ones-column is 0.125 so the
   normalize yields x = 8*O, keeping x in e4m3's normal range; the
   residual qn is pre-scaled by 8 on the host (LayerNorm is
   scale-invariant).
 - Output is fp16 (halves the output DMA); host converts back to fp32.
 - Weight DMAs are hoisted out of the steady-state loop.
 - Emission is ACT-first software pipelining: the S^T head-pair groups of
   seq s (feeding ScalarE's exps) interleave with AV/fc/LN of s-1 and the
   projections of s+1, and carry top scheduler priority so fc/proj
   matmuls never head-of-line-block a ready S^T in the PE queue.
"""

import numpy as np

import concourse.bacc as bacc
import concourse.mybir as mybir
import concourse.tile as tile
from concourse.bass_utils import run_bass_kernel_spmd

F32 = mybir.dt.float32
F16 = mybir.dt.float16
I32 = mybir.dt.int32
BF16 = mybir.dt.bfloat16
F8 = mybir.dt.float8e4
AF = mybir.ActivationFunctionType
OP = mybir.AluOpType
DR = mybir.MatmulPerfMode.DoubleRow

N_CORES = 8
S = 4          # sequences per core
T = 512        # sequence length
D = 512        # model dim (= e = n_head * d_k)
NH = 8         # heads
DV = 64        # head dim
C = 4          # 128-row chunks per 512 dim
P = 128
EPS = 1e-6

_PROGRAM_CACHE = {}


def _build_program(apply_affine: bool, loop_iters: int = 1):
    nc = bacc.Bacc()

    # inputs are host-repacked to [.., P, C*T]: each partition's whole
    # working set is contiguous in DRAM.
    qT = nc.declare_dram_parameter("qT", [S, P, C * T], F8, isOutput=False)
    kT = nc.declare_dram_parameter("kT", [S, P, C * T], F8, isOutput=False)
    vT = nc.declare_dram_parameter("vT", [S, P, C * T], F8, isOutput=False)
    qn = nc.declare_dram_parameter("qn", [S, P, C * D], BF16, isOutput=False)
    wq = nc.declare_dram_parameter("wq", [P, C * D], F8, isOutput=False)
    wk = nc.declare_dram_parameter("wk", [P, C * D], F8, isOutput=False)
    wv = nc.declare_dram_parameter("wv", [P, C * D], F8, isOutput=False)
    wfc = nc.declare_dram_parameter("wfc", [P, C * D], F8, isOutput=False)
    if apply_affine:
        gmb = nc.declare_dram_parameter("gmb", [P, D], F32, isOutput=False)
        btb = nc.declare_dram_parameter("btb", [P, D], F32, isOutput=False)
    out = nc.declare_dram_parameter("out", [S, T, D], BF16, isOutput=True)

    with tile.TileContext(nc) as tc:
        with (
            tc.tile_pool(name="const", bufs=1) as cst,
            tc.tile_pool(name="inp", bufs=3) as inp,
            tc.tile_pool(name="proj", bufs=2) as proj,
            tc.tile_pool(name="expp", bufs=10) as expp,
            tc.tile_pool(name="xp", bufs=3) as xp,
            tc.tile_pool(name="small", bufs=2) as small,
            tc.tile_pool(name="psc", bufs=2, space="PSUM") as psc,
            tc.tile_pool(name="pfc", bufs=2, space="PSUM") as pfc,
            tc.tile_pool(name="pav", bufs=2, space="PSUM") as pavp,
        ):
            wq_sb = cst.tile([P, C, D], F8, tag="wq")
            wk_sb = cst.tile([P, C, D], F8, tag="wk")
            wv_sb = cst.tile([P, C, D], F8, tag="wv")
            wfc_sb = cst.tile([P, C, D], F8, tag="wfc")
            magic_sb = cst.tile([P, 1], I32, tag="magic")
            nc.vector.memset(magic_sb[:], 0x5F3759DF)
            if apply_affine:
                gm_sb = cst.tile([P, D], F32, tag="gmb")
                bt_sb = cst.tile([P, D], F32, tag="btb")
                nc.sync.dma_start(gm_sb[:], gmb[:])
                nc.sync.dma_start(bt_sb[:], btb[:])

            # weights loaded ONCE (outside the steady-state loop)
            for w_sb, w in ((wq_sb, wq), (wk_sb, wk), (wv_sb, wv),
                            (wfc_sb, wfc)):
                nc.sync.dma_start(w_sb.rearrange("p c e -> p (c e)"), w[:])

            def load(s):
                st = {}
                st["qT"] = inp.tile([P, C, T], F8, tag="qT", name="qT_sb")
                st["kT"] = inp.tile([P, C, T], F8, tag="kT", name="kT_sb")
                st["vT"] = inp.tile([P, C, T], F8, tag="vT", name="vT_sb")
                for sb, dr in ((st["qT"], qT), (st["kT"], kT),
                               (st["vT"], vT)):
                    nc.sync.dma_start(
                        sb.rearrange("p c t -> p (c t)"), dr[s]
                    )
                st["qn"] = inp.tile([P, C, D], BF16, tag="qnf", name="qn_sb")
                nc.sync.dma_start(
                    st["qn"].rearrange("p c d -> p (c d)"), qn[s]
                )
                return st

            def projQK(s, st, ecs):
                # Q^T/K^T [e, t] head-major bf16; fp8 DoubleRow matmuls,
                # DVE drains. Chunk ec feeds attnB_group hp=ec.
                if ecs[0] == 0:
                    st["QT"] = proj.tile([P, C, T], BF16, tag="QT",
                                         name="QT_sb")
                    st["KT"] = proj.tile([P, C, T], BF16, tag="KT",
                                         name="KT_sb")
                for ec in ecs:
                    for dst, w_sb, x_sb in (
                        (st["QT"], wq_sb, st["qT"]),
                        (st["KT"], wk_sb, st["kT"]),
                    ):
                        ps = pfc.tile([P, T], F32, tag="fc", name="ps")
                        for dc in range(0, C, 2):
                            nc.tensor.matmul(
                                ps[:],
                                lhsT=w_sb[:, dc:dc + 2, ec * P:(ec + 1) * P],
                                rhs=x_sb[:, dc:dc + 2, :],
                                start=(dc == 0),
                                stop=(dc == C - 2),
                                perf_mode=DR,
                            )
                        nc.vector.tensor_copy(dst[:, ec, :], ps[:])

            def projV(s, st):
                # V [t, e] bf16 with per-head 0.125 column (the AV ones
                # column is 0.125 so x comes out as 8*O for fp8 range)
                V_sb = proj.tile([P, C, NH, DV + 1], BF16, tag="V",
                                 name="V_sb")
                st["V"] = V_sb
                nc.gpsimd.memset(V_sb[:, :, :, DV:DV + 1], 0.125)
                for tc_ in range(C):
                    ps = pfc.tile([P, T], F32, tag="fc", name="ps")
                    for dc in range(0, C, 2):
                        nc.tensor.matmul(
                            ps[:],
                            lhsT=st["vT"][:, dc:dc + 2, tc_ * P:(tc_ + 1) * P],
                            rhs=wv_sb[:, dc:dc + 2, :],
                            start=(dc == 0),
                            stop=(dc == C - 2),
                            perf_mode=DR,
                        )
                    nc.vector.tensor_copy(
                        V_sb[:, tc_, :, 0:DV],
                        ps.rearrange("p (h v) -> p h v", h=NH),
                    )

            def attnB_group(s, st, hp):
                # S^T = K_h Q_h^T for one head pair, bf16; exp(S/8) on
                # ScalarE into bf16 expS tiles. Head pairs share a 2-bank
                # psum tile so one [128,1024] exp drains them. The chain
                # gets top scheduler priority: ScalarE is the binding
                # engine and must never wait behind fc/proj PE work.
                if hp == 0:
                    st["expS"] = []
                eP = expp.tile([P, C, 2, T], BF16, tag="expS", name="expSp")
                st["expS"] += [eP[:, :, 0, :], eP[:, :, 1, :]]
                with tc.high_priority():
                    for tkc in range(C):
                        ps2 = psc.tile([P, 2, T], F32, tag="sc", name="ps2")
                        for sub in range(2):
                            nc.tensor.matmul(
                                ps2[:, sub, :],
                                lhsT=st["KT"][sub * DV:(sub + 1) * DV, hp,
                                              tkc * P:(tkc + 1) * P],
                                rhs=st["QT"][sub * DV:(sub + 1) * DV, hp, :],
                                start=True,
                                stop=True,
                            )
                        nc.scalar.activation(
                            eP[:, tkc, :, :], ps2[:], AF.Exp, scale=0.125,
                        )

            def avH(s, st, half):
                # O-form AV for heads [4*half, 4*half+4): bf16 matmuls; col
                # 64 of each head is sum(w)/8, so the reciprocal-normalize
                # yields x = 8*O, written as fp8 (feeds the fc DoubleRow).
                if half == 0:
                    st["x"] = xp.tile([P, C, T], F8, tag="x", name="x_sb")
                x_sb = st["x"]
                W = DV + 1
                for tqc in range(C):
                    pv = pavp.tile([P, 4 * W], F32, tag="av", name="pav")
                    for hh in range(4):
                        h = 4 * half + hh
                        col = hh * W
                        for tkc in range(C):
                            nc.tensor.matmul(
                                pv[:, col:col + W],
                                lhsT=st["expS"][h][:, tkc,
                                                   tqc * P:(tqc + 1) * P],
                                rhs=st["V"][:, tkc, h, :],
                                start=(tkc == 0),
                                stop=(tkc == C - 1),
                            )
                    rc = small.tile([P, 4], F32, tag="rc", bufs=4, name="rc")
                    nc.vector.reciprocal(rc[:], pv[:, DV:4 * W:W])
                    nc.vector.tensor_tensor(
                        x_sb[:, tqc, half * 256:(half + 1) * 256]
                            .rearrange("p (h v) -> p h v", h=4),
                        pv.rearrange("p (h x) -> p h x", h=4)[:, :, 0:DV],
                        rc[:, :, None].to_broadcast((P, 4, DV)),
                        OP.mult,
                    )

            def tailC_fc(s, st):
                # fc (contracting over the *time* index, thanks to the
                # reference's transpose-view scramble) as fp8 DoubleRow +
                # residual add + LN stats
                st2_seq = small.tile([P, C, 2], F32, tag="st2",
                                     name="st2_seq")
                y_sb = small.tile([P, C, D], F32, tag="y", bufs=2,
                                  name="y_sb")
                y16 = small.tile([P, C, D], BF16, tag="y16", bufs=2,
                                 name="y16")
                st["st2"], st["y"], st["y16"] = st2_seq, y_sb, y16
                for ac in range(C):
                    psy = pfc.tile([P, T], F32, tag="fc", name="psy")
                    for cc in range(0, C, 2):
                        nc.tensor.matmul(
                            psy[:],
                            lhsT=st["x"][:, cc:cc + 2, ac * P:(ac + 1) * P],
                            rhs=wfc_sb[:, cc:cc + 2, :],
                            start=(cc == 0),
                            stop=(cc == C - 2),
                            perf_mode=DR,
                        )
                    nc.vector.tensor_tensor(
                        y_sb[:, ac, :], psy[:], st["qn"][:, ac, :], OP.add
                    )
                    st6 = small.tile([P, 6], F32, tag="st6", name="st6")
                    nc.vector.bn_stats(st6[:], y_sb[:, ac, :])
                    nc.vector.bn_aggr(st2_seq[:, ac, :], st6[:])

            def tailC_ln(s, st):
                st2_seq, y_sb, y16 = st["st2"], st["y"], st["y16"]
                # rinv = rsqrt(var) via the bit-hack seed + 2 Newton
                # iterations, entirely on DVE (max rel err ~5e-6). Keeps
                # Exp as the ONLY ScalarE table.
                rinv = small.tile([P, C], F32, tag="rinv", name="rinv")
                t1 = small.tile([P, C], F32, tag="nt1", name="t1")
                t2 = small.tile([P, C], F32, tag="nt2", name="t2")
                var_i = st2_seq.bitcast(I32)[:, :, 1]
                nc.vector.tensor_scalar(
                    t1.bitcast(I32)[:], var_i, 1, None, OP.arith_shift_right
                )
                nc.vector.tensor_tensor(
                    rinv.bitcast(I32)[:],
                    magic_sb[:].to_broadcast((P, C)),
                    t1.bitcast(I32)[:],
                    OP.subtract,
                )
                for _ in range(2):
                    nc.vector.tensor_tensor(t1[:], rinv[:], rinv[:], OP.mult)
                    nc.vector.tensor_tensor(
                        t2[:], t1[:], st2_seq[:, :, 1], OP.mult
                    )
                    nc.vector.tensor_scalar(
                        t2[:], t2[:], -0.5, 1.5, OP.mult, OP.add
                    )
                    nc.vector.tensor_tensor(rinv[:], rinv[:], t2[:], OP.mult)
                for ac in range(C):
                    if apply_affine:
                        nc.vector.tensor_scalar(
                            y_sb[:, ac, :], y_sb[:, ac, :],
                            st2_seq[:, ac, 0:1], rinv[:, ac:ac + 1],
                            OP.subtract, OP.mult,
                        )
                        nc.vector.tensor_tensor(
                            y_sb[:, ac, :], y_sb[:, ac, :], gm_sb[:], OP.mult
                        )
                        nc.vector.tensor_tensor(
                            y16[:, ac, :], y_sb[:, ac, :], bt_sb[:], OP.add
                        )
                    else:
                        nc.vector.tensor_scalar(
                            y16[:, ac, :], y_sb[:, ac, :],
                            st2_seq[:, ac, 0:1], rinv[:, ac:ac + 1],
                            OP.subtract, OP.mult,
                        )
                # out on the ACT HWDGE ring keeps the SP queue free for
                # the next body's input loads.
                for ac in range(C):
                    nc.scalar.dma_start(
                        out[s, ac * P:(ac + 1) * P, :], y16[:, ac, :]
                    )

            # ACT-first software pipelining (see module docstring)
            def emit_all():
                sts = {}
                sts[0] = load(0)
                sts[1] = load(1)
                projQK(0, sts[0], (0, 1))
                projQK(0, sts[0], (2, 3))
                for s in range(S):
                    if s + 2 < S:
                        sts[s + 2] = load(s + 2)
                    attnB_group(s, sts[s], 0)
                    projV(s, sts[s])
                    if s > 0:
                        avH(s - 1, sts[s - 1], 0)
                    attnB_group(s, sts[s], 1)
                    if s > 0:
                        avH(s - 1, sts[s - 1], 1)
                    attnB_group(s, sts[s], 2)
                    if s > 0:
                        tailC_fc(s - 1, sts[s - 1])
                    if s + 1 < S:
                        projQK(s + 1, sts[s + 1], (0, 1))
                    attnB_group(s, sts[s], 3)
                    if s + 1 < S:
                        projQK(s + 1, sts[s + 1], (2, 3))
                    if s > 0:
                        tailC_ln(s - 1, sts[s - 1])
                avH(S - 1, sts[S - 1], 0)
                avH(S - 1, sts[S - 1], 1)
                tailC_fc(S - 1, sts[S - 1])
                tailC_ln(S - 1, sts[S - 1])

            if loop_iters == 1:
                emit_all()
            else:
                with tc.For_i(0, loop_iters, 1):
                    emit_all()

    nc.finalize()
    return nc


def _get_program(apply_affine: bool, loop_iters: int = 1):
    key = (apply_affine, loop_iters)
    if key not in _PROGRAM_CACHE:
        _PROGRAM_CACHE[key] = _build_program(apply_affine, loop_iters)
    return _PROGRAM_CACHE[key]


def _pack(a, dtype):
    # [.., C*P, F] -> [.., P, C*F] so each partition row is contiguous
    sh = a.shape[:-2]
    cp, f = a.shape[-2], a.shape[-1]
    return np.ascontiguousarray(
        a.reshape(*sh, C, P, f).swapaxes(-3, -2).reshape(*sh, P, C * f)
    ).astype(dtype)


def kernel(q, k, v, w_q, w_k, w_v, w_fc, ln_gamma, ln_beta, _res_holder=None):
    q = np.asarray(q, dtype=np.float32)
    k = np.asarray(k, dtype=np.float32)
    v = np.asarray(v, dtype=np.float32)
    w_q = np.asarray(w_q, dtype=np.float32)
    w_k = np.asarray(w_k, dtype=np.float32)
    w_v = np.asarray(w_v, dtype=np.float32)
    w_fc = np.asarray(w_fc, dtype=np.float32)
    ln_gamma = np.asarray(ln_gamma, dtype=np.float32)
    ln_beta = np.asarray(ln_beta, dtype=np.float32)

    b, n, t, d = q.shape
    B = b * n
    assert (b, n, t, d) == (8, 4, T, D), q.shape
    qf = q.reshape(B, t, d)
    kf = k.reshape(B, t, d)
    vf = v.reshape(B, t, d)

    apply_affine = not (
        np.all(ln_gamma == 1.0) and np.all(ln_beta == 0.0)
    )
    nc = _get_program(apply_affine)

    bf16 = mybir.dt.np(BF16)
    f8 = mybir.dt.np(F8)

    wq_p = _pack(w_q.T, f8)
    wk_p = _pack(w_k.T, f8)
    wv_p = _pack(w_v.T, f8)
    wfc_p = _pack(w_fc.T, f8)

    in_maps = []
    for c in range(N_CORES):
        sl = slice(S * c, S * (c + 1))
        m = {
            "qT": _pack(qf[sl].transpose(0, 2, 1), f8),
            "kT": _pack(kf[sl].transpose(0, 2, 1), f8),
            "vT": _pack(vf[sl].transpose(0, 2, 1), f8),
            "qn": _pack(qf[sl] * 8.0, bf16),
            "wq": wq_p, "wk": wk_p, "wv": wv_p, "wfc": wfc_p,
        }
        if apply_affine:
            m["gmb"] = np.ascontiguousarray(
                np.broadcast_to(ln_gamma, (P, D)).astype(np.float32)
            )
            m["btb"] = np.ascontiguousarray(
                np.broadcast_to(ln_beta, (P, D)).astype(np.float32)
            )
        in_maps.append(m)

    res = run_bass_kernel_spmd(nc, in_maps, list(range(N_CORES)))
    if _res_holder is not None:
        _res_holder.append(res)
    full = np.concatenate(
        [res.results[c]["out"].astype(np.float32) for c in range(N_CORES)],
        axis=0,
    )
    return full.reshape(b, n, t, d)


def prep_per_core(inputs):
    qf = inputs["q"].reshape(32, 512, 512).astype(np.float32)
    kf = inputs["k"].reshape(32, 512, 512).astype(np.float32)
    vf = inputs["v"].reshape(32, 512, 512).astype(np.float32)
    bf16 = mybir.dt.np(BF16)
    f8 = mybir.dt.np(F8)

    wq_p = _pack(inputs["w_q"].T.astype(np.float32), f8)
    wk_p = _pack(inputs["w_k"].T.astype(np.float32), f8)
    wv_p = _pack(inputs["w_v"].T.astype(np.float32), f8)
    wfc_p = _pack(inputs["w_fc"].T.astype(np.float32), f8)

    per_core = []
    for c in range(N_CORES):
        sl = slice(S * c, S * (c + 1))
        per_core.append({
            "qT": _pack(qf[sl].transpose(0, 2, 1), f8),
            "kT": _pack(kf[sl].transpose(0, 2, 1), f8),
            "vT": _pack(vf[sl].transpose(0, 2, 1), f8),
            "qn": _pack(qf[sl] * 8.0, bf16),
            "wq": wq_p, "wk": wk_p, "wv": wv_p, "wfc": wfc_p,
        })
    return per_core


# revision 22
# speedup vs baseline: 1.5581x; 1.0745x over previous
"""Multi-head attention block (QKV proj + SDPA + merge-scramble + fc +
residual + LayerNorm) on 8 Trainium2 NeuronCores.

Sharding: data-parallel over the flattened batch dim (b*n = 32 sequences),
4 sequences per core. Each core runs an identical Bass program on its shard.

v5 = HW-measured optimum of the fp8 design space:
 - Q/K/V projections and the fc matmul run as fp8(e4m3) DoubleRow matmuls
   (2 contraction subtiles per instruction; measured ~1.6x faster than
   bf16 on HW). Inputs q/k/v and weights are quantized to fp8 on the host.
 - S^T = K Q^T stays bf16 (precision: fp8 Q/K noise dominated the error
   budget) and exp keeps its bf16 output (measured: fp8 ACT output costs
   +1.4us per [128,1024] exp -- ~90us/core, a disaster).
 - AV stays bf16 non-DoubleRow (DR would require fp8 expS).
 - x (the attention output) is written as fp8 by the AV-normalize (free on
   DVE) to feed the fc DoubleRow. The V_aug ones-column is 0.125 so the
   normalize yields x = 8*O, keeping x in e4m3's normal range; the
   residual qn is pre-scaled by 8 on the host (LayerNorm is
   scale-invariant).
 - Output is fp16 (halves the output DMA); host converts back to fp32.
 - Weight DMAs are hoisted out of the steady-state loop.
 - Emission is ACT-first software pipelining: the S^T head-pair groups of
   seq s (feeding ScalarE's exps) interleave with AV/fc/LN of s-1 and the
   projections of s+1, and carry top scheduler priority so fc/proj
   matmuls never head-of-line-block a ready S^T in the PE queue.
"""

import numpy as np

import concourse.bacc as bacc
import concourse.mybir as mybir
import concourse.tile as tile
from concourse.bass_utils import run_bass_kernel_spmd

F32 = mybir.dt.float32
F16 = mybir.dt.float16
I32 = mybir.dt.int32
BF16 = mybir.dt.bfloat16
F8 = mybir.dt.float8e4
AF = mybir.ActivationFunctionType
OP = mybir.AluOpType
DR = mybir.MatmulPerfMode.DoubleRow

N_CORES = 8
S = 4          # sequences per core
T = 512        # sequence length
D = 512        # model dim (= e = n_head * d_k)
NH = 8         # heads
DV = 64        # head dim
C = 4          # 128-row chunks per 512 dim
P = 128
EPS = 1e-6

_PROGRAM_CACHE = {}


def _build_program(apply_affine: bool, loop_iters: int = 1):
    nc = bacc.Bacc()

    # inputs are host-repacked to [.., P, C*T]: each partition's whole
    # working set is contiguous in DRAM.
    qT = nc.declare_dram_parameter("qT", [S, P, C * T], F8, isOutput=False)
    kT = nc.declare_dram_parameter("kT", [S, P, C * T], F8, isOutput=False)
    vT = nc.declare_dram_parameter("vT", [S, P, C * T], F8, isOutput=False)
    qn = nc.declare_dram_parameter("qn", [S, P, C * D], BF16, isOutput=False)
    wq = nc.declare_dram_parameter("wq", [P, C * D], F8, isOutput=False)
    wk = nc.declare_dram_parameter("wk", [P, C * D], F8, isOutput=False)
    wv = nc.declare_dram_parameter("wv", [P, C * D], F8, isOutput=False)
    wfc = nc.declare_dram_parameter("wfc", [P, C * D], F8, isOutput=False)
    if apply_affine:
        gmb = nc.declare_dram_parameter("gmb", [P, D], F32, isOutput=False)
        btb = nc.declare_dram_parameter("btb", [P, D], F32, isOutput=False)
    out = nc.declare_dram_parameter("out", [S, T, D], BF16, isOutput=True)

    with tile.TileContext(nc) as tc:
        with (
            tc.tile_pool(name="const", bufs=1) as cst,
            tc.tile_pool(name="inp", bufs=3) as inp,
            tc.tile_pool(name="proj", bufs=2) as proj,
            tc.tile_pool(name="expp", bufs=10) as expp,
            tc.tile_pool(name="xp", bufs=3) as xp,
            tc.tile_pool(name="small", bufs=2) as small,
            tc.tile_pool(name="psc", bufs=2, space="PSUM") as psc,
            tc.tile_pool(name="pfc", bufs=2, space="PSUM") as pfc,
            tc.tile_pool(name="pav", bufs=2, space="PSUM") as pavp,
        ):
            wq_sb = cst.tile([P, C, D], F8, tag="wq")
            wk_sb = cst.tile([P, C, D], F8, tag="wk")
            wv_sb = cst.tile([P, C, D], F8, tag="wv")
            wfc_sb = cst.tile([P, C, D], F8, tag="wfc")
            magic_sb = cst.tile([P, 1], I32, tag="magic")
            nc.vector.memset(magic_sb[:], 0x5F3759DF)
            if apply_affine:
                gm_sb = cst.tile([P, D], F32, tag="gmb")
                bt_sb = cst.tile([P, D], F32, tag="btb")
                nc.sync.dma_start(gm_sb[:], gmb[:])
                nc.sync.dma_start(bt_sb[:], btb[:])

            # weights loaded ONCE (outside the steady-state loop)
            for w_sb, w in ((wq_sb, wq), (wk_sb, wk), (wv_sb, wv),
                            (wfc_sb, wfc)):
                nc.sync.dma_start(w_sb.rearrange("p c e -> p (c e)"), w[:])

            def load(s):
                st = {}
                st["qT"] = inp.tile([P, C, T], F8, tag="qT", name="qT_sb")
                st["kT"] = inp.tile([P, C, T], F8, tag="kT", name="kT_sb")
                st["vT"] = inp.tile([P, C, T], F8, tag="vT", name="vT_sb")
                for sb, dr in ((st["qT"], qT), (st["kT"], kT),
                               (st["vT"], vT)):
                    nc.sync.dma_start(
                        sb.rearrange("p c t -> p (c t)"), dr[s]
                    )
                st["qn"] = inp.tile([P, C, D], BF16, tag="qnf", name="qn_sb")
                nc.sync.dma_start(
                    st["qn"].rearrange("p c d -> p (c d)"), qn[s]
                )
                return st

            def projQK(s, st, ecs):
                # Q^T/K^T [e, t] head-major bf16; fp8 DoubleRow matmuls,
                # DVE drains. Chunk ec feeds attnB_group hp=ec.
                if ecs[0] == 0:
                    st["QT"] = proj.tile([P, C, T], BF16, tag="QT",
                                         name="QT_sb")
                    st["KT"] = proj.tile([P, C, T], BF16, tag="KT",
                                         name="KT_sb")
                for ec in ecs:
                    for dst, w_sb, x_sb in (
                        (st["QT"], wq_sb, st["qT"]),
                        (st["KT"], wk_sb, st["kT"]),
                    ):
                        ps = pfc.tile([P, T], F32, tag="fc", name="ps")
                        for dc in range(0, C, 2):
                            nc.tensor.matmul(
                                ps[:],
                                lhsT=w_sb[:, dc:dc + 2, ec * P:(ec + 1) * P],
                                rhs=x_sb[:, dc:dc + 2, :],
                                start=(dc == 0),
                                stop=(dc == C - 2),
                                perf_mode=DR,
                            )
                        nc.vector.tensor_copy(dst[:, ec, :], ps[:])

            def projV(s, st):
                # V [t, e] bf16 with per-head 0.125 column (the AV ones
                # column is 0.125 so x comes out as 8*O for fp8 range)
                V_sb = proj.tile([P, C, NH, DV + 1], BF16, tag="V",
                                 name="V_sb")
                st["V"] = V_sb
                nc.gpsimd.memset(V_sb[:, :, :, DV:DV + 1], 0.125)
                for tc_ in range(C):
                    ps = pfc.tile([P, T], F32, tag="fc", name="ps")
                    for dc in range(0, C, 2):
                        nc.tensor.matmul(
                            ps[:],
                            lhsT=st["vT"][:, dc:dc + 2, tc_ * P:(tc_ + 1) * P],
                            rhs=wv_sb[:, dc:dc + 2, :],
                            start=(dc == 0),
                            stop=(dc == C - 2),
                            perf_mode=DR,
                        )
                    nc.vector.tensor_copy(
                        V_sb[:, tc_, :, 0:DV],
                        ps.rearrange("p (h v) -> p h v", h=NH),
                    )

            def attnB_group(s, st, hp):
                # S^T = K_h Q_h^T for one head pair, bf16; exp(S/8) on
                # ScalarE into bf16 expS tiles. Head pairs share a 2-bank
                # psum tile so one [128,1024] exp drains them. The chain
                # gets top scheduler priority: ScalarE is the binding
                # engine and must never wait behind fc/proj PE work.
                if hp == 0:
                    st["expS"] = []
                eP = expp.tile([P, C, 2, T], BF16, tag="expS", name="expSp")
                st["expS"] += [eP[:, :, 0, :], eP[:, :, 1, :]]
                with tc.high_priority():
                    for tkc in range(C):
                        ps2 = psc.tile([P, 2, T], F32, tag="sc", name="ps2")
                        for sub in range(2):
                            nc.tensor.matmul(
                                ps2[:, sub, :],
                                lhsT=st["KT"][sub * DV:(sub + 1) * DV, hp,
                                              tkc * P:(tkc + 1) * P],
                                rhs=st["QT"][sub * DV:(sub + 1) * DV, hp, :],
                                start=True,
                                stop=True,
                            )
                        nc.scalar.activation(
                            eP[:, tkc, :, :], ps2[:], AF.Exp, scale=0.125,
                        )

            def avH(s, st, half):
                # O-form AV for heads [4*half, 4*half+4): bf16 matmuls; col
                # 64 of each head is sum(w)/8, so the reciprocal-normalize
                # yields x = 8*O, written as fp8 (feeds the fc DoubleRow).
                if half == 0:
                    st["x"] = xp.tile([P, C, T], F8, tag="x", name="x_sb")
                x_sb = st["x"]
                W = DV + 1
                for tqc in range(C):
                    pv = pavp.tile([P, 4 * W], F32, tag="av", name="pav")
                    for hh in range(4):
                        h = 4 * half + hh
                        col = hh * W
                        for tkc in range(C):
                            nc.tensor.matmul(
                                pv[:, col:col + W],
                                lhsT=st["expS"][h][:, tkc,
                                                   tqc * P:(tqc + 1) * P],
                                rhs=st["V"][:, tkc, h, :],
                                start=(tkc == 0),
                                stop=(tkc == C - 1),
                            )
                    rc = small.tile([P, 4], F32, tag="rc", bufs=4, name="rc")
                    nc.vector.reciprocal(rc[:], pv[:, DV:4 * W:W])
                    nc.vector.tensor_tensor(
                        x_sb[:, tqc, half * 256:(half + 1) * 256]
                            .rearrange("p (h v) -> p h v", h=4),
                        pv.rearrange("p (h x) -> p h x", h=4)[:, :, 0:DV],
                        rc[:, :, None].to_broadcast((P, 4, DV)),
                        OP.mult,
                    )

            def tailC_fc(s, st):
                # fc (contracting over the *time* index, thanks to the
                # reference's transpose-view scramble) as fp8 DoubleRow +
                # residual add + LN stats
                st2_seq = small.tile([P, C, 2], F32, tag="st2",
                                     name="st2_seq")
                y_sb = small.tile([P, C, D], BF16, tag="y", bufs=2,
                                  name="y_sb")
                y16 = small.tile([P, C, D], BF16, tag="y16", bufs=2,
                                 name="y16")
                st["st2"], st["y"], st["y16"] = st2_seq, y_sb, y16
                for ac in range(C):
                    psy = pfc.tile([P, T], F32, tag="fc", name="psy")
                    for cc in range(0, C, 2):
                        nc.tensor.matmul(
                            psy[:],
                            lhsT=st["x"][:, cc:cc + 2, ac * P:(ac + 1) * P],
                            rhs=wfc_sb[:, cc:cc + 2, :],
                            start=(cc == 0),
                            stop=(cc == C - 2),
                            perf_mode=DR,
                        )
                    nc.vector.tensor_tensor(
                        y_sb[:, ac, :], psy[:], st["qn"][:, ac, :], OP.add
                    )
                    st6 = small.tile([P, 6], F32, tag="st6", name="st6")
                    nc.vector.bn_stats(st6[:], y_sb[:, ac, :])
                    nc.vector.bn_aggr(st2_seq[:, ac, :], st6[:])

            def tailC_ln(s, st):
                st2_seq, y_sb, y16 = st["st2"], st["y"], st["y16"]
                # rinv = rsqrt(var) via the bit-hack seed + 2 Newton
                # iterations, entirely on DVE (max rel err ~5e-6). Keeps
                # Exp as the ONLY ScalarE table.
                rinv = small.tile([P, C], F32, tag="rinv", name="rinv")
                t1 = small.tile([P, C], F32, tag="nt1", name="t1")
                t2 = small.tile([P, C], F32, tag="nt2", name="t2")
                var_i = st2_seq.bitcast(I32)[:, :, 1]
                nc.vector.tensor_scalar(
                    t1.bitcast(I32)[:], var_i, 1, None, OP.arith_shift_right
                )
                nc.vector.tensor_tensor(
                    rinv.bitcast(I32)[:],
                    magic_sb[:].to_broadcast((P, C)),
                    t1.bitcast(I32)[:],
                    OP.subtract,
                )
                for _ in range(2):
                    nc.vector.tensor_tensor(t1[:], rinv[:], rinv[:], OP.mult)
                    nc.vector.tensor_tensor(
                        t2[:], t1[:], st2_seq[:, :, 1], OP.mult
                    )
                    nc.vector.tensor_scalar(
                        t2[:], t2[:], -0.5, 1.5, OP.mult, OP.add
                    )
                    nc.vector.tensor_tensor(rinv[:], rinv[:], t2[:], OP.mult)
                for ac in range(C):
                    if apply_affine:
                        nc.vector.tensor_scalar(
                            y_sb[:, ac, :], y_sb[:, ac, :],
                            st2_seq[:, ac, 0:1], rinv[:, ac:ac + 1],
                            OP.subtract, OP.mult,
                        )
                        nc.vector.tensor_tensor(
                            y_sb[:, ac, :], y_sb[:, ac, :], gm_sb[:], OP.mult
                        )
                        nc.vector.tensor_tensor(
                            y16[:, ac, :], y_sb[:, ac, :], bt_sb[:], OP.add
                        )
                    else:
                        nc.vector.tensor_scalar(
                            y16[:, ac, :], y_sb[:, ac, :],
                            st2_seq[:, ac, 0:1], rinv[:, ac:ac + 1],
                            OP.subtract, OP.mult,
                        )
                # out on the ACT HWDGE ring keeps the SP queue free for
                # the next body's input loads.
                for ac in range(C):
                    nc.scalar.dma_start(
                        out[s, ac * P:(ac + 1) * P, :], y16[:, ac, :]
                    )

            # ACT-first software pipelining (see module docstring)
            def emit_all():
                sts = {}
                sts[0] = load(0)
                sts[1] = load(1)
                projQK(0, sts[0], (0, 1))
                projQK(0, sts[0], (2, 3))
                for s in range(S):
                    if s + 2 < S:
                        sts[s + 2] = load(s + 2)
                    attnB_group(s, sts[s], 0)
                    projV(s, sts[s])
                    if s > 0:
                        avH(s - 1, sts[s - 1], 0)
                    attnB_group(s, sts[s], 1)
                    if s > 0:
                        avH(s - 1, sts[s - 1], 1)
                    attnB_group(s, sts[s], 2)
                    if s > 0:
                        tailC_fc(s - 1, sts[s - 1])
                    if s + 1 < S:
                        projQK(s + 1, sts[s + 1], (0, 1))
                    attnB_group(s, sts[s], 3)
                    if s + 1 < S:
                        projQK(s + 1, sts[s + 1], (2, 3))
                    if s > 0:
                        tailC_ln(s - 1, sts[s - 1])
                avH(S - 1, sts[S - 1], 0)
                avH(S - 1, sts[S - 1], 1)
                tailC_fc(S - 1, sts[S - 1])
                tailC_ln(S - 1, sts[S - 1])

            if loop_iters == 1:
                emit_all()
            else:
                with tc.For_i(0, loop_iters, 1):
                    emit_all()

    nc.finalize()
    return nc


def _get_program(apply_affine: bool, loop_iters: int = 1):
    key = (apply_affine, loop_iters)
    if key not in _PROGRAM_CACHE:
        _PROGRAM_CACHE[key] = _build_program(apply_affine, loop_iters)
    return _PROGRAM_CACHE[key]


def _pack(a, dtype):
    # [.., C*P, F] -> [.., P, C*F] so each partition row is contiguous
    sh = a.shape[:-2]
    cp, f = a.shape[-2], a.shape[-1]
    return np.ascontiguousarray(
        a.reshape(*sh, C, P, f).swapaxes(-3, -2).reshape(*sh, P, C * f)
    ).astype(dtype)


def kernel(q, k, v, w_q, w_k, w_v, w_fc, ln_gamma, ln_beta, _res_holder=None):
    q = np.asarray(q, dtype=np.float32)
    k = np.asarray(k, dtype=np.float32)
    v = np.asarray(v, dtype=np.float32)
    w_q = np.asarray(w_q, dtype=np.float32)
    w_k = np.asarray(w_k, dtype=np.float32)
    w_v = np.asarray(w_v, dtype=np.float32)
    w_fc = np.asarray(w_fc, dtype=np.float32)
    ln_gamma = np.asarray(ln_gamma, dtype=np.float32)
    ln_beta = np.asarray(ln_beta, dtype=np.float32)

    b, n, t, d = q.shape
    B = b * n
    assert (b, n, t, d) == (8, 4, T, D), q.shape
    qf = q.reshape(B, t, d)
    kf = k.reshape(B, t, d)
    vf = v.reshape(B, t, d)

    apply_affine = not (
        np.all(ln_gamma == 1.0) and np.all(ln_beta == 0.0)
    )
    nc = _get_program(apply_affine)

    bf16 = mybir.dt.np(BF16)
    f8 = mybir.dt.np(F8)

    wq_p = _pack(w_q.T, f8)
    wk_p = _pack(w_k.T, f8)
    wv_p = _pack(w_v.T, f8)
    wfc_p = _pack(w_fc.T, f8)

    in_maps = []
    for c in range(N_CORES):
        sl = slice(S * c, S * (c + 1))
        m = {
            "qT": _pack(qf[sl].transpose(0, 2, 1), f8),
            "kT": _pack(kf[sl].transpose(0, 2, 1), f8),
            "vT": _pack(vf[sl].transpose(0, 2, 1), f8),
            "qn": _pack(qf[sl] * 8.0, bf16),
            "wq": wq_p, "wk": wk_p, "wv": wv_p, "wfc": wfc_p,
        }
        if apply_affine:
            m["gmb"] = np.ascontiguousarray(
                np.broadcast_to(ln_gamma, (P, D)).astype(np.float32)
            )
            m["btb"] = np.ascontiguousarray(
                np.broadcast_to(ln_beta, (P, D)).astype(np.float32)
            )
        in_maps.append(m)

    res = run_bass_kernel_spmd(nc, in_maps, list(range(N_CORES)))
    if _res_holder is not None:
        _res_holder.append(res)
    full = np.concatenate(
        [res.results[c]["out"].astype(np.float32) for c in range(N_CORES)],
        axis=0,
    )
    return full.reshape(b, n, t, d)


def prep_per_core(inputs):
    qf = inputs["q"].reshape(32, 512, 512).astype(np.float32)
    kf = inputs["k"].reshape(32, 512, 512).astype(np.float32)
    vf = inputs["v"].reshape(32, 512, 512).astype(np.float32)
    bf16 = mybir.dt.np(BF16)
    f8 = mybir.dt.np(F8)

    wq_p = _pack(inputs["w_q"].T.astype(np.float32), f8)
    wk_p = _pack(inputs["w_k"].T.astype(np.float32), f8)
    wv_p = _pack(inputs["w_v"].T.astype(np.float32), f8)
    wfc_p = _pack(inputs["w_fc"].T.astype(np.float32), f8)

    per_core = []
    for c in range(N_CORES):
        sl = slice(S * c, S * (c + 1))
        per_core.append({
            "qT": _pack(qf[sl].transpose(0, 2, 1), f8),
            "kT": _pack(kf[sl].transpose(0, 2, 1), f8),
            "vT": _pack(vf[sl].transpose(0, 2, 1), f8),
            "qn": _pack(qf[sl] * 8.0, bf16),
            "wq": wq_p, "wk": wk_p, "wv": wv_p, "wfc": wfc_p,
        })
    return per_core
